# revision 12
# baseline (speedup 1.0000x reference)
"""MoE routing kernel for 8 Trainium2 NeuronCores.

Problem: B=65536 tokens, shared Linear(512->256)+ReLU, then per-token expert
MLP Linear(256->100)+ReLU -> Linear(100->1), expert chosen by idx in [0,16).

Strategy (expert-parallel, host-side routing):
  - Host sorts tokens by expert. Experts 2c and 2c+1 go to core c, each in a
    fixed-capacity slot of C tokens (C = max expert count rounded up to 8),
    padded with token 0 (padding outputs are computed then discarded).
  - Host pre-transposes x to [512, TOK] bf16 per core so the contraction dim
    (IN_DIM) lands on SBUF partitions: all three GEMMs then chain on-chip with
    no transposes (layer-1 out [hid, tok] feeds layer-2, which feeds layer-3).
  - Device: per group of <=512 tokens: one DMA of x columns, 8 accumulating
    matmuls (512-dim contraction, 2 hid chunks) + bias/ReLU, 2 matmuls for
    expert FC1 + bias/ReLU, 1 matmul for FC2 + bias (b2 folded via a ones row
    of h1), fc2 psum shared by 4 groups (partitions 0/32/64/96), then one
    copy + one DMA out per 4-group block.
  - DMA queues: x rides the sync HWDGE queue exclusively (posted first so
    group 0 lands ASAP); weights ride the scalar HWDGE queue in parallel;
    output blocks ride the gpsimd software DGE.
  - Weights (tiny) live resident in SBUF in bf16; PSUM accumulates fp32.
"""

import math
import os
import sys

import numpy as np

for _p in ("/opt/trn_rl_repo", "/opt/pypackages"):
    if _p not in sys.path and os.path.isdir(_p):
        sys.path.append(_p)

import ml_dtypes

BF16 = ml_dtypes.bfloat16

B, IN_DIM, HID, EXP_HID, OUT_DIM, N_EXP = 65536, 512, 256, 100, 1, 16
N_CORES = 8
GROUP = 512  # tokens per matmul group (= PSUM bank free-dim in fp32)

_PROGRAM_CACHE = {}


def _block_schedule(C: int):
    """Execution-order blocks: (exec_idx, expert_slot, token_offset, ntok).

    Slot A's short remainder block first (smallest x DMA lands earliest),
    slot B's remainder last (short tail chain)."""
    n_full = C // GROUP
    r = C % GROUP
    blocks = []
    if r:
        blocks.append((0, r))
    blocks += [(0, GROUP)] * n_full + [(1, GROUP)] * n_full
    if r:
        blocks.append((1, r))
    out = []
    off = [0, 0]
    # offsets: slot A remainder lives at the end of slot A's range, but we
    # execute it first; just assign offsets in per-slot execution order.
    for slot, n in blocks:
        out.append((len(out), slot, slot * C + off[slot], n))
        off[slot] += n
    return out


def _build_program(C: int):
    """Build (and cache) the Bass program for per-expert-slot capacity C."""
    import concourse.bass as bass
    import concourse.mybir as mybir
    import concourse.tile as tile
    from concourse import bacc

    TOK = 2 * C
    f32 = mybir.dt.float32
    bf16 = mybir.dt.bfloat16
    AF = mybir.ActivationFunctionType
    ALU = mybir.AluOpType

    nc = bacc.Bacc("TRN2", target_bir_lowering=False, debug=False)

    groups = _block_schedule(C)
    n_groups = len(groups)
    n_blk = (n_groups + 2) // 3

    # x pre-blocked on host in EXECUTION order:
    # xg[g, p, kc*512+t] = x[token of exec-group g at pos t, kc*128+p]
    xg_d = nc.dram_tensor(
        "xg", [n_groups, 128, 4 * GROUP], bf16, kind="ExternalInput"
    ).ap()
    # ws pre-packed p-major on host: ws[p, kc*HID + m] = Ws[kc*128+p, m], so
    # the DMA is fully contiguous (2KB rows) instead of 512B gather packets
    ws_d = nc.dram_tensor("ws", [128, 4 * HID], bf16, kind="ExternalInput").ap()
    bs_d = nc.dram_tensor("bs", [128, 2], f32, kind="ExternalInput").ap()
    # w1 padded to 128 output cols (100 real) so FWL kicks in on LDWEIGHTS
    w1_d = nc.dram_tensor("w1", [128, 2 * 2 * 128], bf16, kind="ExternalInput").ap()
    # b1 rows 0..99 = b1[e]; rows 100..127 = 1.0 so relu(0 + 1) makes a ones
    # row block that w2's bias row consumes (fc2 bias folded into the matmul)
    b1_d = nc.dram_tensor("b1", [128, 2], f32, kind="ExternalInput").ap()
    # w2 rows 0..99 = W2[e,:,0], row 100 = b2[e], rows 101..127 = 0
    w2_d = nc.dram_tensor("w2", [128, 2], bf16, kind="ExternalInput").ap()
    # out[blk, lane, t]: exec-group g = blk*4+lane, token t of that group
    out_d = nc.dram_tensor("out", [n_blk, 3, GROUP], f32, kind="ExternalOutput").ap()

    with tile.TileContext(nc) as tc:
        with (
            tc.tile_pool(name="const", bufs=1) as const,
            tc.tile_pool(name="xp", bufs=8) as xp,
            tc.tile_pool(name="hp", bufs=3) as hp,
            tc.tile_pool(name="h1p", bufs=3) as h1p,
            tc.tile_pool(name="ob", bufs=2) as obp,
            tc.tile_pool(name="ps1", bufs=4, space="PSUM") as ps1,
            tc.tile_pool(name="ps2", bufs=2, space="PSUM") as ps2,
            tc.tile_pool(name="ps3", bufs=2, space="PSUM") as ps3,
        ):
            ws_sb = const.tile([128, 4, HID], bf16)
            bs_sb = const.tile([128, 2], f32)
            w1_sb = const.tile([128, 2, 2, 128], bf16)
            b1_sb = const.tile([128, 2], f32)
            w2_sb = const.tile([128, 2], bf16)
            warm_w = const.tile([128, 128], bf16)
            x_tiles = []

            # All x tiles up front (allocation is metadata-only; the DMA
            # start is what gets scheduled). All x groups ride the sync
            # HWDGE queue, unsplit (one ~650ns post per group keeps the
            # supply rate at ~330GB/s).
            for i, (bi, _, _, _) in enumerate(groups):
                x_tiles.append(
                    xp.tile([128, 4, GROUP], bf16, tag="x", name=f"x_sb{bi}")
                )

            def post_x(i, eng):
                bi, _, _, n = groups[i]
                x_sb = x_tiles[i]
                if n == GROUP:
                    eng.dma_start(x_sb.rearrange("p c t -> p (c t)"), xg_d[bi])
                else:
                    xg_v = xg_d[bi].rearrange("p (c t) -> p c t", c=4)
                    eng.dma_start(x_sb[:, :, :n], xg_v[:, :, :n])

            # Layer-1 weights ride the sync queue FIRST (contiguous 2KB rows,
            # ~0.9us). Expert weights + biases (needed one group later) ride
            # the gpsimd software DGE, off the x queue.
            nc.sync.dma_start(ws_sb.rearrange("p c m -> p (c m)"), ws_d)
            nc.gpsimd.dma_start(
                w1_sb.rearrange("p e c m -> p (e c m)"), w1_d
            )
            nc.gpsimd.dma_start(bs_sb[:, :], bs_d[:, :])
            nc.gpsimd.dma_start(b1_sb[:, :], b1_d[:, :])
            nc.gpsimd.dma_start(w2_sb[:, :], w2_d[:, :])
            for i in range(len(groups)):
                post_x(i, nc.sync)

            # PE warm-up: short matmuls while the first x DMA is in flight,
            # keeping the PE clock ramp going. Results are never read.
            nc.vector.memset(warm_w[:, :], 0.0)
            warm_p = ps1.tile([128, GROUP], f32, tag="p1", name="warm_p")
            for _ in range(6):
                nc.tensor.matmul(
                    warm_p[:, :128], warm_w[:, :], warm_w[:, :], start=True, stop=True
                )

            # Two-deep software pipeline over the PE stream: iteration i runs
            # L1(i), fc1(i-1), fc2(i-2). Every activation (h, h1) then has
            # >=1.5us between its producing ReLU and its consuming matmul, so
            # the in-order PE stream never stalls on the Vector/Scalar
            # engines.
            h_tiles = {}
            h1_tiles = {}
            p3 = None

            def do_l1(i):
                _, _, _, n = groups[i]
                x_sb = x_tiles[i]
                h_sb = hp.tile([128, 2, GROUP], bf16, tag="h")
                for hc in range(2):
                    p1 = ps1.tile([128, GROUP], f32, tag="p1")
                    for kc in range(4):
                        nc.tensor.matmul(
                            p1[:, :n],
                            ws_sb[:, kc, hc * 128 : (hc + 1) * 128],
                            x_sb[:, kc, :n],
                            start=(kc == 0),
                            stop=(kc == 3),
                        )
                    # h = relu(psum + bs): hc0 on VectorE, hc1 on ScalarE
                    if hc == 0:
                        nc.vector.tensor_scalar(
                            h_sb[:, hc, :n],
                            p1[:, :n],
                            bs_sb[:, hc : hc + 1],
                            0.0,
                            ALU.add,
                            ALU.max,
                        )
                    else:
                        nc.scalar.activation(
                            h_sb[:, hc, :n],
                            p1[:, :n],
                            AF.Relu,
                            bias=bs_sb[:, hc : hc + 1],
                        )
                h_tiles[i] = h_sb

            def do_fc1(j):
                if j < 0 or j >= len(groups):
                    return
                _, e, _, n = groups[j]
                h_sb = h_tiles.pop(j)
                p2 = ps2.tile([128, GROUP], f32, tag="p2")
                for kc in range(2):
                    nc.tensor.matmul(
                        p2[:, :n],
                        w1_sb[:, e, kc, :],
                        h_sb[:, kc, :n],
                        start=(kc == 0),
                        stop=(kc == 1),
                    )
                # h1 rows 0..99 = relu(psum + b1); rows 100..127 = relu(0+1)=1
                # alternate engines so neither Vector nor Scalar is loaded 2x
                h1_sb = h1p.tile([128, GROUP], bf16, tag="h1")
                if j % 2 == 0:
                    nc.vector.tensor_scalar(
                        h1_sb[:, :n],
                        p2[:, :n],
                        b1_sb[:, e : e + 1],
                        0.0,
                        ALU.add,
                        ALU.max,
                    )
                else:
                    nc.scalar.activation(
                        h1_sb[:, :n],
                        p2[:, :n],
                        AF.Relu,
                        bias=b1_sb[:, e : e + 1],
                    )
                h1_tiles[j] = h1_sb

            def issue_fc2(j):
                nonlocal p3
                if j < 0 or j >= len(groups):
                    return
                _, e, _, n = groups[j]
                # 3 exec-groups share one PSUM bank at partitions 0/32/64
                # (PE col-tile base must be 0/32/64)
                lane = j % 3
                if lane == 0:
                    p3 = ps3.tile([128, GROUP], f32, tag="p3")
                nc.tensor.matmul(
                    p3[32 * lane : 32 * lane + 1, :n],
                    w2_sb[:, e : e + 1],
                    h1_tiles.pop(j)[:, :n],
                    start=True,
                    stop=True,
                )
                if lane == 2 or j == len(groups) - 1:
                    blk = j // 3
                    ob = obp.tile([128, GROUP], f32, tag="ob")
                    nc.scalar.copy(ob[:, :], p3[:, :])
                    # rows 0/32/64 of ob hold the 3 lanes' fc2 outputs
                    ob_lanes = ob.rearrange("(l s) t -> l s t", s=32)[:, 0, :]
                    nc.sync.dma_start(
                        out_d[blk, : lane + 1, :], ob_lanes[: lane + 1, :]
                    )

            for i in range(len(groups)):
                do_l1(i)
                do_fc1(i - 1)
                issue_fc2(i - 2)
            do_fc1(len(groups) - 1)
            issue_fc2(len(groups) - 2)
            issue_fc2(len(groups) - 1)

    nc.compile()
    return nc


def _get_program(C: int):
    if C not in _PROGRAM_CACHE:
        _PROGRAM_CACHE[C] = _build_program(C)
    return _PROGRAM_CACHE[C]


def kernel(x, idx, Ws, bs, W1, b1, W2, b2, _trace=False, _result_box=None):
    from concourse.bass_utils import run_bass_kernel_spmd

    x = np.asarray(x)
    idx = np.asarray(idx).astype(np.int64)
    Ws = np.asarray(Ws, dtype=np.float32)
    bs = np.asarray(bs, dtype=np.float32)
    W1 = np.asarray(W1, dtype=np.float32)
    b1 = np.asarray(b1, dtype=np.float32)
    W2 = np.asarray(W2, dtype=np.float32)
    b2 = np.asarray(b2, dtype=np.float32)

    counts = np.bincount(idx, minlength=N_EXP)
    C = max(GROUP, int(math.ceil(counts.max() / 8) * 8))
    nc = _get_program(C)
    groups = _block_schedule(C)
    n_groups = len(groups)
    n_blk = (n_groups + 2) // 3

    order = np.argsort(idx, kind="stable")
    bounds = np.zeros(N_EXP + 1, dtype=np.int64)
    np.cumsum(counts, out=bounds[1:])
    tok_by_expert = [order[bounds[e] : bounds[e + 1]] for e in range(N_EXP)]

    # shared-layer weights, chunked for the device (same for every core)
    # p-major: ws_host[p, kc, m] = Ws[kc*128+p, m]
    ws_host = np.ascontiguousarray(
        Ws.reshape(4, 128, HID).transpose(1, 0, 2).reshape(128, 4 * HID)
    ).astype(BF16)
    bs_host = np.ascontiguousarray(bs.reshape(2, 128).T).astype(np.float32)

    x_bf = x.astype(BF16)
    in_maps = []
    core_tokens = []
    for c in range(N_CORES):
        ea, eb = 2 * c, 2 * c + 1
        # per-slot token lists padded to C with token 0
        toks = np.zeros(2 * C, dtype=np.int64)
        toks[: counts[ea]] = tok_by_expert[ea]
        toks[C : C + counts[eb]] = tok_by_expert[eb]
        core_tokens.append(toks)

        # execution-order groups: gather each group's tokens
        toks_p = np.zeros(n_groups * GROUP, dtype=np.int64)
        for g, (_, slot, off, n) in enumerate(groups):
            toks_p[g * GROUP : g * GROUP + n] = toks[off : off + n]
        xg = np.ascontiguousarray(
            x_bf[toks_p].reshape(n_groups, GROUP, 4, 128).transpose(0, 3, 2, 1)
        ).reshape(n_groups, 128, 4 * GROUP)

        w1_pair = np.zeros((2, 2, 128, 128), dtype=BF16)
        w1_pair[:, :, :, :EXP_HID] = W1[[ea, eb]].reshape(2, 2, 128, EXP_HID).astype(BF16)
        # p-major contiguous: w1_pair[p, e, kc, m]
        w1_pair = np.ascontiguousarray(w1_pair.transpose(2, 0, 1, 3)).reshape(
            128, 2 * 2 * 128
        )
        b1_pair = np.ones((128, 2), dtype=np.float32)
        b1_pair[:EXP_HID] = b1[[ea, eb]].T
        w2_pair = np.zeros((128, 2), dtype=BF16)
        w2_pair[:EXP_HID] = W2[[ea, eb], :, 0].T.astype(BF16)
        w2_pair[EXP_HID] = b2[[ea, eb], 0].astype(BF16)

        in_maps.append(
            {
                "xg": xg,
                "ws": ws_host,
                "bs": bs_host,
                "w1": w1_pair,
                "b1": b1_pair,
                "w2": w2_pair,
            }
        )

    res = run_bass_kernel_spmd(
        nc,
        in_maps,
        core_ids=list(range(N_CORES)),
        trace=_trace,
        **({"trace_cores": [0]} if _trace else {}),
    )
    if _result_box is not None:
        _result_box.append(res)

    out = np.zeros((B, OUT_DIM), dtype=np.float32)
    for c in range(N_CORES):
        ea, eb = 2 * c, 2 * c + 1
        oc = res.results[c]["out"].reshape(n_blk * 3, GROUP)  # exec-group major
        # scatter back: group g's cols [0, n) are slot tokens [off, off+n)
        vals = np.zeros(2 * C, dtype=np.float32)
        for g, (_, slot, off, n) in enumerate(groups):
            vals[off : off + n] = oc[g, :n]
        out[core_tokens[c][: counts[ea]], 0] = vals[: counts[ea]]
        out[core_tokens[c][C : C + counts[eb]], 0] = vals[C : C + counts[eb]]
    return out


# revision 14
# speedup vs baseline: 1.0371x; 1.0371x over previous
"""MoE routing kernel for 8 Trainium2 NeuronCores.

Problem: B=65536 tokens, shared Linear(512->256)+ReLU, then per-token expert
MLP Linear(256->100)+ReLU -> Linear(100->1), expert chosen by idx in [0,16).

Strategy (expert-parallel, host-side routing):
  - Host sorts tokens by expert. Experts 2c and 2c+1 go to core c, each in a
    fixed-capacity slot of C tokens (C = max expert count rounded up to 8),
    padded with token 0 (padding outputs are computed then discarded).
  - Host pre-transposes x to [512, TOK] bf16 per core so the contraction dim
    (IN_DIM) lands on SBUF partitions: all three GEMMs then chain on-chip with
    no transposes (layer-1 out [hid, tok] feeds layer-2, which feeds layer-3).
  - Device: per group of <=512 tokens: one DMA of x columns, 8 accumulating
    matmuls (512-dim contraction, 2 hid chunks) + bias/ReLU, 2 matmuls for
    expert FC1 + bias/ReLU, 1 matmul for FC2 + bias (b2 folded via a ones row
    of h1), fc2 psum shared by 4 groups (partitions 0/32/64/96), then one
    copy + one DMA out per 4-group block.
  - DMA queues: x rides the sync HWDGE queue exclusively (posted first so
    group 0 lands ASAP); weights ride the scalar HWDGE queue in parallel;
    output blocks ride the gpsimd software DGE.
  - Weights (tiny) live resident in SBUF in bf16; PSUM accumulates fp32.
"""

import math
import os
import sys

import numpy as np

for _p in ("/opt/trn_rl_repo", "/opt/pypackages"):
    if _p not in sys.path and os.path.isdir(_p):
        sys.path.append(_p)

import ml_dtypes

BF16 = ml_dtypes.bfloat16

B, IN_DIM, HID, EXP_HID, OUT_DIM, N_EXP = 65536, 512, 256, 100, 1, 16
N_CORES = 8
GROUP = 512  # tokens per matmul group (= PSUM bank free-dim in fp32)

_PROGRAM_CACHE = {}


def _block_schedule(C: int):
    """Execution-order blocks: (exec_idx, expert_slot, token_offset, ntok).

    Slot A's short remainder block first (smallest x DMA lands earliest),
    slot B's remainder last (short tail chain)."""
    n_full = C // GROUP
    r = C % GROUP
    blocks = []
    if r:
        blocks.append((0, r))
    blocks += [(0, GROUP)] * n_full + [(1, GROUP)] * n_full
    if r:
        blocks.append((1, r))
    out = []
    off = [0, 0]
    # offsets: slot A remainder lives at the end of slot A's range, but we
    # execute it first; just assign offsets in per-slot execution order.
    for slot, n in blocks:
        out.append((len(out), slot, slot * C + off[slot], n))
        off[slot] += n
    return out


def _build_program(C: int):
    """Build (and cache) the Bass program for per-expert-slot capacity C."""
    import concourse.bass as bass
    import concourse.mybir as mybir
    import concourse.tile as tile
    from concourse import bacc

    TOK = 2 * C
    f32 = mybir.dt.float32
    bf16 = mybir.dt.bfloat16
    AF = mybir.ActivationFunctionType
    ALU = mybir.AluOpType

    nc = bacc.Bacc("TRN2", target_bir_lowering=False, debug=False)

    groups = _block_schedule(C)
    n_groups = len(groups)
    n_blk = (n_groups + 2) // 3

    # x pre-blocked on host in EXECUTION order:
    # xg[g, p, kc*512+t] = x[token of exec-group g at pos t, kc*128+p]
    xg_d = nc.dram_tensor(
        "xg", [n_groups, 128, 4 * GROUP], bf16, kind="ExternalInput"
    ).ap()
    # ws pre-packed p-major on host: ws[p, kc*HID + m] = Ws[kc*128+p, m], so
    # the DMA is fully contiguous (2KB rows) instead of 512B gather packets
    ws_d = nc.dram_tensor("ws", [128, 4 * HID], bf16, kind="ExternalInput").ap()
    bs_d = nc.dram_tensor("bs", [128, 2], f32, kind="ExternalInput").ap()
    # w1 padded to 128 output cols (100 real) so FWL kicks in on LDWEIGHTS
    w1_d = nc.dram_tensor("w1", [128, 2 * 2 * 128], bf16, kind="ExternalInput").ap()
    # b1 rows 0..99 = b1[e]; rows 100..127 = 1.0 so relu(0 + 1) makes a ones
    # row block that w2's bias row consumes (fc2 bias folded into the matmul)
    b1_d = nc.dram_tensor("b1", [128, 2], f32, kind="ExternalInput").ap()
    # w2 rows 0..99 = W2[e,:,0], row 100 = b2[e], rows 101..127 = 0
    w2_d = nc.dram_tensor("w2", [128, 2], bf16, kind="ExternalInput").ap()
    # out[blk, lane, t]: exec-group g = blk*4+lane, token t of that group
    out_d = nc.dram_tensor("out", [n_blk, 3, GROUP], f32, kind="ExternalOutput").ap()

    with tile.TileContext(nc) as tc:
        with (
            tc.tile_pool(name="const", bufs=1) as const,
            tc.tile_pool(name="xp", bufs=8) as xp,
            tc.tile_pool(name="hp", bufs=3) as hp,
            tc.tile_pool(name="h1p", bufs=3) as h1p,
            tc.tile_pool(name="ob", bufs=2) as obp,
            tc.tile_pool(name="ps1", bufs=4, space="PSUM") as ps1,
            tc.tile_pool(name="ps2", bufs=2, space="PSUM") as ps2,
            tc.tile_pool(name="ps3", bufs=2, space="PSUM") as ps3,
        ):
            ws_sb = const.tile([128, 4, HID], bf16)
            bs_sb = const.tile([128, 2], f32)
            w1_sb = const.tile([128, 2, 2, 128], bf16)
            b1_sb = const.tile([128, 2], f32)
            w2_sb = const.tile([128, 2], bf16)
            warm_w = const.tile([128, GROUP], bf16)
            x_tiles = []

            # All x tiles up front (allocation is metadata-only; the DMA
            # start is what gets scheduled). All x groups ride the sync
            # HWDGE queue, unsplit (one ~650ns post per group keeps the
            # supply rate at ~330GB/s).
            for i, (bi, _, _, _) in enumerate(groups):
                x_tiles.append(
                    xp.tile([128, 4, GROUP], bf16, tag="x", name=f"x_sb{bi}")
                )

            def post_x(i, eng):
                bi, _, _, n = groups[i]
                x_sb = x_tiles[i]
                if n == GROUP:
                    eng.dma_start(x_sb.rearrange("p c t -> p (c t)"), xg_d[bi])
                else:
                    xg_v = xg_d[bi].rearrange("p (c t) -> p c t", c=4)
                    eng.dma_start(x_sb[:, :, :n], xg_v[:, :, :n])

            # Layer-1 weights ride the sync queue FIRST (contiguous 2KB rows,
            # ~0.9us). Expert weights + biases (needed one group later) ride
            # the gpsimd software DGE, off the x queue.
            nc.sync.dma_start(ws_sb.rearrange("p c m -> p (c m)"), ws_d)
            nc.gpsimd.dma_start(
                w1_sb.rearrange("p e c m -> p (e c m)"), w1_d
            )
            nc.gpsimd.dma_start(bs_sb[:, :], bs_d[:, :])
            nc.gpsimd.dma_start(b1_sb[:, :], b1_d[:, :])
            nc.gpsimd.dma_start(w2_sb[:, :], w2_d[:, :])
            for i in range(len(groups)):
                post_x(i, nc.sync)

            # PE warm-up: full-width matmuls that keep the PE continuously
            # busy from program start until the first x tile lands, so the
            # PE clock (p-state) is fully ramped when real work begins. An
            # idle PE drops back to half speed (~427ns/512-col matmul).
            # Results are never read.
            nc.vector.memset(warm_w[:, :], 0.0)
            warm_p = ps1.tile([128, GROUP], f32, tag="p1", name="warm_p")
            for _ in range(9):
                nc.tensor.matmul(
                    warm_p[:, :], warm_w[:, :128], warm_w[:, :], start=True, stop=True
                )

            # Two-deep software pipeline over the PE stream: iteration i runs
            # L1(i), fc1(i-1), fc2(i-2). Every activation (h, h1) then has
            # >=1.5us between its producing ReLU and its consuming matmul, so
            # the in-order PE stream never stalls on the Vector/Scalar
            # engines.
            h_tiles = {}
            h1_tiles = {}
            p3 = None

            def do_l1(i):
                _, _, _, n = groups[i]
                x_sb = x_tiles[i]
                h_sb = hp.tile([128, 2, GROUP], bf16, tag="h")
                for hc in range(2):
                    p1 = ps1.tile([128, GROUP], f32, tag="p1")
                    for kc in range(4):
                        nc.tensor.matmul(
                            p1[:, :n],
                            ws_sb[:, kc, hc * 128 : (hc + 1) * 128],
                            x_sb[:, kc, :n],
                            start=(kc == 0),
                            stop=(kc == 3),
                        )
                    # h = relu(psum + bs): hc0 on VectorE, hc1 on ScalarE
                    if hc == 0:
                        nc.vector.tensor_scalar(
                            h_sb[:, hc, :n],
                            p1[:, :n],
                            bs_sb[:, hc : hc + 1],
                            0.0,
                            ALU.add,
                            ALU.max,
                        )
                    else:
                        nc.scalar.activation(
                            h_sb[:, hc, :n],
                            p1[:, :n],
                            AF.Relu,
                            bias=bs_sb[:, hc : hc + 1],
                        )
                h_tiles[i] = h_sb

            def do_fc1(j):
                if j < 0 or j >= len(groups):
                    return
                _, e, _, n = groups[j]
                h_sb = h_tiles.pop(j)
                p2 = ps2.tile([128, GROUP], f32, tag="p2")
                for kc in range(2):
                    nc.tensor.matmul(
                        p2[:, :n],
                        w1_sb[:, e, kc, :],
                        h_sb[:, kc, :n],
                        start=(kc == 0),
                        stop=(kc == 1),
                    )
                # h1 rows 0..99 = relu(psum + b1); rows 100..127 = relu(0+1)=1
                # alternate engines so neither Vector nor Scalar is loaded 2x
                h1_sb = h1p.tile([128, GROUP], bf16, tag="h1")
                if j % 2 == 0:
                    nc.vector.tensor_scalar(
                        h1_sb[:, :n],
                        p2[:, :n],
                        b1_sb[:, e : e + 1],
                        0.0,
                        ALU.add,
                        ALU.max,
                    )
                else:
                    nc.scalar.activation(
                        h1_sb[:, :n],
                        p2[:, :n],
                        AF.Relu,
                        bias=b1_sb[:, e : e + 1],
                    )
                h1_tiles[j] = h1_sb

            def issue_fc2(j):
                nonlocal p3
                if j < 0 or j >= len(groups):
                    return
                _, e, _, n = groups[j]
                # 3 exec-groups share one PSUM bank at partitions 0/32/64
                # (PE col-tile base must be 0/32/64)
                lane = j % 3
                if lane == 0:
                    p3 = ps3.tile([128, GROUP], f32, tag="p3")
                nc.tensor.matmul(
                    p3[32 * lane : 32 * lane + 1, :n],
                    w2_sb[:, e : e + 1],
                    h1_tiles.pop(j)[:, :n],
                    start=True,
                    stop=True,
                )
                if lane == 2 or j == len(groups) - 1:
                    blk = j // 3
                    ob = obp.tile([128, GROUP], f32, tag="ob")
                    nc.scalar.copy(ob[:, :], p3[:, :])
                    # rows 0/32/64 of ob hold the 3 lanes' fc2 outputs
                    ob_lanes = ob.rearrange("(l s) t -> l s t", s=32)[:, 0, :]
                    nc.sync.dma_start(
                        out_d[blk, : lane + 1, :], ob_lanes[: lane + 1, :]
                    )

            for i in range(len(groups)):
                do_l1(i)
                do_fc1(i - 1)
                issue_fc2(i - 2)
            do_fc1(len(groups) - 1)
            issue_fc2(len(groups) - 2)
            issue_fc2(len(groups) - 1)

    nc.compile()
    return nc


def _get_program(C: int):
    if C not in _PROGRAM_CACHE:
        _PROGRAM_CACHE[C] = _build_program(C)
    return _PROGRAM_CACHE[C]


def kernel(x, idx, Ws, bs, W1, b1, W2, b2, _trace=False, _result_box=None):
    from concourse.bass_utils import run_bass_kernel_spmd

    x = np.asarray(x)
    idx = np.asarray(idx).astype(np.int64)
    Ws = np.asarray(Ws, dtype=np.float32)
    bs = np.asarray(bs, dtype=np.float32)
    W1 = np.asarray(W1, dtype=np.float32)
    b1 = np.asarray(b1, dtype=np.float32)
    W2 = np.asarray(W2, dtype=np.float32)
    b2 = np.asarray(b2, dtype=np.float32)

    counts = np.bincount(idx, minlength=N_EXP)
    C = max(GROUP, int(math.ceil(counts.max() / 8) * 8))
    nc = _get_program(C)
    groups = _block_schedule(C)
    n_groups = len(groups)
    n_blk = (n_groups + 2) // 3

    order = np.argsort(idx, kind="stable")
    bounds = np.zeros(N_EXP + 1, dtype=np.int64)
    np.cumsum(counts, out=bounds[1:])
    tok_by_expert = [order[bounds[e] : bounds[e + 1]] for e in range(N_EXP)]

    # shared-layer weights, chunked for the device (same for every core)
    # p-major: ws_host[p, kc, m] = Ws[kc*128+p, m]
    ws_host = np.ascontiguousarray(
        Ws.reshape(4, 128, HID).transpose(1, 0, 2).reshape(128, 4 * HID)
    ).astype(BF16)
    bs_host = np.ascontiguousarray(bs.reshape(2, 128).T).astype(np.float32)

    x_bf = x.astype(BF16)
    in_maps = []
    core_tokens = []
    for c in range(N_CORES):
        ea, eb = 2 * c, 2 * c + 1
        # per-slot token lists padded to C with token 0
        toks = np.zeros(2 * C, dtype=np.int64)
        toks[: counts[ea]] = tok_by_expert[ea]
        toks[C : C + counts[eb]] = tok_by_expert[eb]
        core_tokens.append(toks)

        # execution-order groups: gather each group's tokens
        toks_p = np.zeros(n_groups * GROUP, dtype=np.int64)
        for g, (_, slot, off, n) in enumerate(groups):
            toks_p[g * GROUP : g * GROUP + n] = toks[off : off + n]
        xg = np.ascontiguousarray(
            x_bf[toks_p].reshape(n_groups, GROUP, 4, 128).transpose(0, 3, 2, 1)
        ).reshape(n_groups, 128, 4 * GROUP)

        w1_pair = np.zeros((2, 2, 128, 128), dtype=BF16)
        w1_pair[:, :, :, :EXP_HID] = W1[[ea, eb]].reshape(2, 2, 128, EXP_HID).astype(BF16)
        # p-major contiguous: w1_pair[p, e, kc, m]
        w1_pair = np.ascontiguousarray(w1_pair.transpose(2, 0, 1, 3)).reshape(
            128, 2 * 2 * 128
        )
        b1_pair = np.ones((128, 2), dtype=np.float32)
        b1_pair[:EXP_HID] = b1[[ea, eb]].T
        w2_pair = np.zeros((128, 2), dtype=BF16)
        w2_pair[:EXP_HID] = W2[[ea, eb], :, 0].T.astype(BF16)
        w2_pair[EXP_HID] = b2[[ea, eb], 0].astype(BF16)

        in_maps.append(
            {
                "xg": xg,
                "ws": ws_host,
                "bs": bs_host,
                "w1": w1_pair,
                "b1": b1_pair,
                "w2": w2_pair,
            }
        )

    res = run_bass_kernel_spmd(
        nc,
        in_maps,
        core_ids=list(range(N_CORES)),
        trace=_trace,
        **({"trace_cores": [0]} if _trace else {}),
    )
    if _result_box is not None:
        _result_box.append(res)

    out = np.zeros((B, OUT_DIM), dtype=np.float32)
    for c in range(N_CORES):
        ea, eb = 2 * c, 2 * c + 1
        oc = res.results[c]["out"].reshape(n_blk * 3, GROUP)  # exec-group major
        # scatter back: group g's cols [0, n) are slot tokens [off, off+n)
        vals = np.zeros(2 * C, dtype=np.float32)
        for g, (_, slot, off, n) in enumerate(groups):
            vals[off : off + n] = oc[g, :n]
        out[core_tokens[c][: counts[ea]], 0] = vals[: counts[ea]]
        out[core_tokens[c][C : C + counts[eb]], 0] = vals[C : C + counts[eb]]
    return out
